# revision 19
# baseline (speedup 1.0000x reference)
"""Trainium2 Bass kernel for the BiaffineLayer problem.

Math (per batch b):
  out[l, m, c] = x1[l] @ W1[c] + x2[m] @ W2[c]
              + sum_h x1[l,h] * x2[m,h] * W3[c,h]
              + sum_h |x1[l,h] - x2[m,h]| * W4[c,h] + bias[c]
  shapes: x1, x2 [2, 512, 128]; W [25, 512]; bias [25]; out [2, 512, 512, 25]

Sharding: 8 cores = 2 batches x 4 m-blocks of 128 columns. Core (b, mb) gets
full x1[b] and its x2[b, m0:m0+128] block; it produces out[b, :, m0:m0+MB, :].

Decomposition, with |d| = 2*relu(d) - d and d = x1 - x2[m]:
  out = x1t' @ V3  +  D_m' @ (2 W4T)  +  T2B
where (host-precomputed except D):
  V3[h,(m,c)] = x2[m,h]*W3[c,h] + (W1-W4)[c,h]   (t3 + t1 - t4's -x1*W4 part)
  T2B[m,c]    = x2[m] @ (W2+W4)T + b             (added on the HOST during
                                                  unshard - pure per-(m,c))
  D_m[h,l]    = relu(x1[l,h] - x2[m,h])          (device, pairwise)

Measured HW rates (microbenched): DVE tensor_scalar = 129ns + 0.26ns/elem
(~263ns per [128,512] D tile, 2x bf16 mode already engaged); ACT relu ~612ns
effective, ACT [128,1600] PSUM drain ~1480ns; PE t4 LDW+MM pair 25ns, t3
400-col MM 168ns (PE total ~18us, not critical); input DMA is descriptor-
rate-bound (~17ns per 1KB partition row -> 2.2us for x1t on one queue);
gpsimd compute useless (7.5us/op); large out-DMAs on the gpsimd queue hit a
~19us descriptor pathology - keep them on sync.

Schedule (v3):
  - input DMAs split across queues to parallelize descriptor processing:
    sync: x1t[:,0:256] -> wv3a(w4t2+v3 blocks 0-1) -> out-DMAs
    vector: x1t[:,256:512] -> wv3b(v3 blocks 2-4)
    gpsimd: negx2[:,0:64] -> negx2[:,64:128] -> wv3c(v3 blocks 5-7)
    D-gen starts ~9.1us instead of ~10.6.
  - single 4-bank PSUM tile per block (bufs=2), drain = ONE ACT op
    [128,4,400] per block, emitted one block late; last block's drain
    split DVE+ACT with its two out-DMAs on sync+tensor queues.
  - D-tile engine split DVE 102 / ACT 26 (block 0-1 give ACT 4 tiles as
    they carry no/less drain work).
"""

import sys

sys.path.insert(0, "/opt/trn_rl_repo")

from contextlib import ExitStack

import ml_dtypes
import numpy as np

import concourse.bass as bass
import concourse.tile as tile
from concourse import bacc, bass_utils, mybir

F32 = mybir.dt.float32
BF16 = mybir.dt.bfloat16
BF16_NP = ml_dtypes.bfloat16

B, L, H, C = 2, 512, 128, 25
MB = 128            # m-block per core
N_CORES = 8
MSUB = 16           # m's per psum block
N_MS = MB // MSUB   # 8 blocks over the m-block
LCHUNK = 128
N_LC = L // LCHUNK  # 4 l-chunks
CHUNK_F = MSUB * C  # 400 psum free columns per l-chunk slice
PS_STRIDE = 512     # psum bank stride (f32 elems) per l-chunk slice

# Per-block D-tile engine split: V=DVE, A=ACT. sum V = 108.
# ACT's per-block queue is [A-tiles..., drain(k-1)]; A positions are EARLY
# (j=1,3,..) so the block's end never waits on the slower ACT engine --
# the cadence is then set purely by DVE (nV x 263ns). The drain runs in
# ACT's idle tail of each block. First and last blocks are ACT-heavy:
# block 0 has no drain yet, and a DVE-light final block shortens the tail
# (DVE finishes early and takes half of the final drain).
D_PATS = {
    12: "VAVAVAVAVVVVVVVV",   # V=12 A=4 at j=1,3,5,7
    14: "VAVAVVVVVVVVVVVV",   # V=14 A=2 at j=1,3
}
D_NV = [12, 14, 14, 14, 14, 14, 14, 12]   # sum = 108


def build_kernel(nc: bass.Bass, repeat: int = 1):
    # xin = x1t bf16 [H, 512] cols 0:512 | negx2 f32 [H, 128] as bf16-viewed
    # bytes in cols 512:768. Merging rides negx2's bytes on x1t's fat
    # 1536B rows: one DMA per partition-half at full bandwidth instead of
    # a separate 512B-row DMA that measured ~5x slower and landed last.
    xin = nc.dram_tensor("xin", (H, L + 2 * MB), BF16, kind="ExternalInput").ap()
    # w4t2 (25 cols) | v3 (3200 cols) merged
    wv3 = nc.dram_tensor("wv3", (H, C + MB * C), BF16, kind="ExternalInput").ap()
    out = nc.dram_tensor("out", (L, MB * C), BF16, kind="ExternalOutput").ap()

    WVA = C + 2 * CHUNK_F        # w4t2 + v3 blocks 0-1

    with tile.TileContext(nc) as tc, ExitStack() as ctx:
      const = ctx.enter_context(tc.tile_pool(name="const", bufs=1))
      dpool = ctx.enter_context(tc.tile_pool(name="dpool", bufs=128))
      opool = ctx.enter_context(tc.tile_pool(name="opool", bufs=10))
      psum = ctx.enter_context(tc.tile_pool(name="psum", bufs=2, space="PSUM"))
      for _rep in range(repeat):
        # ---- input loads ----
        # DMA queues are sync/scalar/gpsimd only. A DMA costs ~1.3us
        # fixed startup + transfer at ~230GB/s for >=1KB rows, and a
        # ring's SECOND DMA starts ~1us after its first finishes. So
        # everything D-gen needs rides FIRST on a ring: xin halves
        # (partition split) on sync+scalar, w4t2+v3[0:2] on gpsimd.
        # The scalar ring processes its half while the ACT engine loads
        # the activation table (act_warm), costing ACT nothing.
        xin_tile = const.tile([H, L + 2 * MB], BF16)
        x1t_bf = xin_tile[:, 0:L]
        negx2_f = xin_tile[:, L:].bitcast(F32)
        wv3_tile = const.tile([H, C + MB * C], BF16)
        w4t2_bf = wv3_tile[:, 0:C]
        v3_bf = wv3_tile[:, C:]
        ones_bf = const.tile([1, LCHUNK], BF16)

        nc.scalar.dma_start(xin_tile[86:128, :], xin[86:128, :])
        nc.sync.dma_start(xin_tile[0:43, :], xin[0:43, :])
        nc.gpsimd.dma_start(xin_tile[43:86, :], xin[43:86, :])
        nc.gpsimd.dma_start(wv3_tile[:, 0:WVA], wv3[:, 0:WVA])
        nc.vector.memset(ones_bf[:], 1.0)
        # preload the ACT activation table off the critical path
        act_warm = const.tile([1, LCHUNK], BF16)
        nc.scalar.activation(act_warm[:], ones_bf[:],
                             mybir.ActivationFunctionType.Relu)

        # ---- main loop over m-blocks ----
        # Drains (one ACT op each) emitted one block late so they queue
        # behind the next block's D work, prioritizing D production.
        pend = None

        def emit_drain(p):
            ms_, ps3_, last = p
            out3 = (out[:, ms_ * CHUNK_F : (ms_ + 1) * CHUNK_F]
                    .rearrange("(lc p) c -> p lc c", p=LCHUNK))
            if not last:
                o_sb = opool.tile([LCHUNK, N_LC * CHUNK_F], BF16)
                o3 = o_sb[:].rearrange("p (lc c) -> p lc c", c=CHUNK_F)
                nc.scalar.copy(o3[:], ps3_)
                nc.sync.dma_start(out3, o3)
            else:  # split engines + 2 DMAs on 2 queues for a short tail;
                # separate tiles so the two halves never serialize on a
                # same-tile write dependency
                oa = opool.tile([LCHUNK, 2 * CHUNK_F], BF16)
                ob = opool.tile([LCHUNK, 2 * CHUNK_F], BF16)
                oa3 = oa[:].rearrange("p (lc c) -> p lc c", c=CHUNK_F)
                ob3 = ob[:].rearrange("p (lc c) -> p lc c", c=CHUNK_F)
                nc.vector.tensor_copy(oa3[:], ps3_[:, 0:2])
                nc.sync.dma_start(out3[:, 0:2], oa3)
                nc.scalar.copy(ob3[:], ps3_[:, 2:4])
                nc.scalar.dma_start(out3[:, 2:4], ob3)

        for ms in range(N_MS):
            # D tiles for this block
            dts = []
            pat = D_PATS[D_NV[ms]]
            for j in range(MSUB):
                m = ms * MSUB + j
                dt_ = dpool.tile([H, L], BF16, tag="d")
                if pat[j] == "V":
                    nc.vector.tensor_scalar(
                        dt_[:], x1t_bf, negx2_f[:, m : m + 1], 0.0,
                        op0=mybir.AluOpType.add, op1=mybir.AluOpType.max)
                else:
                    nc.scalar.activation(
                        dt_[:], x1t_bf, mybir.ActivationFunctionType.Relu,
                        bias=negx2_f[:, m : m + 1], scale=1.0)
                dts.append(dt_)

            if ms == 0:
                # v3 bulk rides second on the sync ring; lands ~14us,
                # needed by block 2's t3 (~17.5us)
                nc.sync.dma_start(wv3_tile[:, WVA:], wv3[:, WVA:])
            if pend is not None:
                emit_drain(pend)

            ps = psum.tile([LCHUNK, N_LC * PS_STRIDE], F32, tag="ps")
            # t3 (+t1 fold) opens each group full-width (PSUM zeroing is
            # bank-granular), then the t4s accumulate j-major.
            for lc in range(N_LC):
                nc.tensor.matmul(
                    ps[:, lc * PS_STRIDE : lc * PS_STRIDE + CHUNK_F],
                    x1t_bf[:, lc * LCHUNK : (lc + 1) * LCHUNK],
                    v3_bf[:, ms * CHUNK_F : (ms + 1) * CHUNK_F],
                    start=True, stop=False, skip_group_check=True)
            for j in range(MSUB):
                for lc in range(N_LC):
                    base = lc * PS_STRIDE
                    nc.tensor.matmul(
                        ps[:, base + j * C : base + (j + 1) * C],
                        dts[j][:, lc * LCHUNK : (lc + 1) * LCHUNK],
                        w4t2_bf[:],
                        start=False, stop=(j == MSUB - 1),
                        skip_group_check=True)

            ps3 = ps[:].rearrange("p (lc x) -> p lc x",
                                  x=PS_STRIDE)[:, :, 0:CHUNK_F]
            pend = (ms, ps3, ms == N_MS - 1)
        emit_drain(pend)
    return nc


_COMPILED = {}


def _get_compiled():
    if "nc" not in _COMPILED:
        nc = bacc.Bacc("TRN2", target_bir_lowering=False, debug=False,
                       num_devices=N_CORES)
        build_kernel(nc)
        nc.compile()
        _COMPILED["nc"] = nc
    return _COMPILED["nc"]


def make_in_maps(x1, x2, W, b):
    W1, W2, W3, W4 = (W[:, 0:H], W[:, H : 2 * H], W[:, 2 * H : 3 * H],
                      W[:, 3 * H : 4 * H])
    w13 = (W1 - W4).T.astype(np.float32)          # [H, C]
    w3t = W3.T.astype(np.float32)                 # [H, C]
    w4t2 = (2.0 * W4).T.astype(np.float32)        # [H, C]
    in_maps = []
    for cid in range(N_CORES):
        bb, mblk = cid // 4, cid % 4
        m0 = mblk * MB
        x2blk = x2[bb, m0 : m0 + MB]              # [MB, H]
        x2t = x2blk.T                             # [H, MB]
        # V3[h, m*C+c] = x2t[h,m]*W3T[h,c] + (W1-W4)T[h,c]
        v3 = x2t[:, :, None] * w3t[:, None, :] + w13[:, None, :]
        wv3 = np.concatenate([w4t2, v3.reshape(H, MB * C)], axis=1)
        # xin: x1t bf16 cols 0:512 | negx2 f32 bytes viewed as bf16 cols
        # 512:768 (bitcast back to f32 on device)
        xin = np.empty((H, L + 2 * MB), dtype=BF16_NP)
        xin[:, 0:L] = x1[bb].T.astype(BF16_NP)
        negx2 = np.ascontiguousarray(-x2t.astype(np.float32))
        xin[:, L:] = negx2.view(np.uint16).view(BF16_NP).reshape(H, 2 * MB)
        in_maps.append({
            "xin": xin,
            "wv3": np.ascontiguousarray(wv3.astype(BF16_NP)),
        })
    return in_maps


def t2_bias(x2, W, b):
    """Host-side t2 term: x2 @ (W2+W4).T + bias, [B, L, C] f32."""
    W2 = W[:, H : 2 * H]
    W4 = W[:, 3 * H : 4 * H]
    return (x2 @ (W2 + W4).T + b).astype(np.float32)


def run_on_device(x1, x2, W, b, trace=False, trace_kwargs=None):
    nc = _get_compiled()
    in_maps = make_in_maps(x1, x2, W, b)
    res = bass_utils.run_bass_kernel_spmd(
        nc, in_maps, core_ids=list(range(N_CORES)), trace=trace,
        **(trace_kwargs or {}))
    t2 = t2_bias(x2, W, b)                        # [B, L, C]
    full = np.empty((B, L, L, C), dtype=np.float32)
    for cid in range(N_CORES):
        bb, mblk = cid // 4, cid % 4
        m0 = mblk * MB
        full[bb, :, m0 : m0 + MB, :] = (
            np.asarray(res.results[cid]["out"])
            .astype(np.float32).reshape(L, MB, C)
            + t2[bb, m0 : m0 + MB, :][None, :, :])
    return full, res


def kernel(x1, x2, W, b):
    x1 = np.asarray(x1, dtype=np.float32)
    x2 = np.asarray(x2, dtype=np.float32)
    W = np.asarray(W, dtype=np.float32)
    b = np.asarray(b, dtype=np.float32)
    full, _ = run_on_device(x1, x2, W, b, trace=False)
    return full


# revision 20
# speedup vs baseline: 1.1139x; 1.1139x over previous
"""Trainium2 Bass kernel for the BiaffineLayer problem.

Math (per batch b):
  out[l, m, c] = x1[l] @ W1[c] + x2[m] @ W2[c]
              + sum_h x1[l,h] * x2[m,h] * W3[c,h]
              + sum_h |x1[l,h] - x2[m,h]| * W4[c,h] + bias[c]
  shapes: x1, x2 [2, 512, 128]; W [25, 512]; bias [25]; out [2, 512, 512, 25]

Sharding: 8 cores = 2 batches x 4 m-blocks of 128 columns. Core (b, mb) gets
full x1[b] and its x2[b, m0:m0+128] block; it produces out[b, :, m0:m0+MB, :].

Decomposition, with |d| = 2*relu(d) - d and d = x1 - x2[m]:
  out = x1t' @ V3  +  D_m' @ (2 W4T)  +  T2B
where (host-precomputed except D):
  V3[h,(m,c)] = x2[m,h]*W3[c,h] + (W1-W4)[c,h]   (t3 + t1 - t4's -x1*W4 part)
  T2B[m,c]    = x2[m] @ (W2+W4)T + b             (added on the HOST during
                                                  unshard - pure per-(m,c))
  D_m[h,l]    = relu(x1[l,h] - x2[m,h])          (device, pairwise)

Measured HW rates (microbenched): DVE tensor_scalar = 129ns + 0.26ns/elem
(~263ns per [128,512] D tile, 2x bf16 mode already engaged); ACT relu ~612ns
effective, ACT [128,1600] PSUM drain ~1480ns; PE t4 LDW+MM pair 25ns, t3
400-col MM 168ns (PE total ~18us, not critical); input DMA is descriptor-
rate-bound (~17ns per 1KB partition row -> 2.2us for x1t on one queue);
gpsimd compute useless (7.5us/op); large out-DMAs on the gpsimd queue hit a
~19us descriptor pathology - keep them on sync.

Schedule (v3):
  - input DMAs split across queues to parallelize descriptor processing:
    sync: x1t[:,0:256] -> wv3a(w4t2+v3 blocks 0-1) -> out-DMAs
    vector: x1t[:,256:512] -> wv3b(v3 blocks 2-4)
    gpsimd: negx2[:,0:64] -> negx2[:,64:128] -> wv3c(v3 blocks 5-7)
    D-gen starts ~9.1us instead of ~10.6.
  - single 4-bank PSUM tile per block (bufs=2), drain = ONE ACT op
    [128,4,400] per block, emitted one block late; last block's drain
    split DVE+ACT with its two out-DMAs on sync+tensor queues.
  - D-tile engine split DVE 102 / ACT 26 (block 0-1 give ACT 4 tiles as
    they carry no/less drain work).
"""

import sys

sys.path.insert(0, "/opt/trn_rl_repo")

from contextlib import ExitStack

import ml_dtypes
import numpy as np

import concourse.bass as bass
import concourse.tile as tile
from concourse import bacc, bass_utils, mybir

F32 = mybir.dt.float32
BF16 = mybir.dt.bfloat16
BF16_NP = ml_dtypes.bfloat16

B, L, H, C = 2, 512, 128, 25
MB = 128            # m-block per core
N_CORES = 8
MSUB = 16           # m's per psum block
N_MS = MB // MSUB   # 8 blocks over the m-block
LCHUNK = 128
N_LC = L // LCHUNK  # 4 l-chunks
CHUNK_F = MSUB * C  # 400 psum free columns per l-chunk slice
PS_STRIDE = 512     # psum bank stride (f32 elems) per l-chunk slice

# Per-block D-tile engine split: V=DVE, A=ACT. sum V = 108.
# ACT's per-block queue is [A-tiles..., drain(k-1)]; A positions are EARLY
# (j=1,3,..) so the block's end never waits on the slower ACT engine --
# the cadence is then set purely by DVE (nV x 263ns). The drain runs in
# ACT's idle tail of each block. First and last blocks are ACT-heavy:
# block 0 has no drain yet, and a DVE-light final block shortens the tail
# (DVE finishes early and takes half of the final drain).
D_PATS = {
    12: "VAVAVAVAVVVVVVVV",   # V=12 A=4 at j=1,3,5,7
    14: "VAVAVVVVVVVVVVVV",   # V=14 A=2 at j=1,3
}
D_NV = [12, 14, 14, 14, 14, 14, 14, 12]   # sum = 108


def build_kernel(nc: bass.Bass, repeat: int = 1):
    # xin = x1t bf16 [H, 512] cols 0:512 | negx2 f32 [H, 128] as bf16-viewed
    # bytes in cols 512:768. Merging rides negx2's bytes on x1t's fat
    # 1536B rows: one DMA per partition-half at full bandwidth instead of
    # a separate 512B-row DMA that measured ~5x slower and landed last.
    xin = nc.dram_tensor("xin", (H, L + 2 * MB), BF16, kind="ExternalInput").ap()
    # w4t2 (25 cols) | v3 (3200 cols) merged
    wv3 = nc.dram_tensor("wv3", (H, C + MB * C), BF16, kind="ExternalInput").ap()
    out = nc.dram_tensor("out", (L, MB * C), BF16, kind="ExternalOutput").ap()

    WVA = C + 2 * CHUNK_F        # w4t2 + v3 blocks 0-1

    with tile.TileContext(nc) as tc, ExitStack() as ctx:
      const = ctx.enter_context(tc.tile_pool(name="const", bufs=1))
      dpool = ctx.enter_context(tc.tile_pool(name="dpool", bufs=128))
      opool = ctx.enter_context(tc.tile_pool(name="opool", bufs=10))
      psum = ctx.enter_context(tc.tile_pool(name="psum", bufs=2, space="PSUM"))
      for _rep in range(repeat):
        # ---- input loads ----
        # DMA queues are sync/scalar/gpsimd only. A DMA costs ~1.3us
        # fixed startup + transfer at ~230GB/s for >=1KB rows, and a
        # ring's SECOND DMA starts ~1us after its first finishes. So
        # everything D-gen needs rides FIRST on a ring: xin halves
        # (partition split) on sync+scalar, w4t2+v3[0:2] on gpsimd.
        # The scalar ring processes its half while the ACT engine loads
        # the activation table (act_warm), costing ACT nothing.
        xin_tile = const.tile([H, L + 2 * MB], BF16)
        x1t_bf = xin_tile[:, 0:L]
        negx2_f = xin_tile[:, L:].bitcast(F32)
        wv3_tile = const.tile([H, C + MB * C], BF16)
        w4t2_bf = wv3_tile[:, 0:C]
        v3_bf = wv3_tile[:, C:]
        ones_bf = const.tile([1, LCHUNK], BF16)

        nc.scalar.dma_start(xin_tile[64:128, :], xin[64:128, :])
        nc.sync.dma_start(xin_tile[0:64, :], xin[0:64, :])
        nc.gpsimd.dma_start(wv3_tile[:, 0:WVA], wv3[:, 0:WVA])
        nc.vector.memset(ones_bf[:], 1.0)
        # preload the ACT activation table off the critical path
        act_warm = const.tile([1, LCHUNK], BF16)
        nc.scalar.activation(act_warm[:], ones_bf[:],
                             mybir.ActivationFunctionType.Relu)

        # ---- main loop over m-blocks ----
        # Drains (one ACT op each) emitted one block late so they queue
        # behind the next block's D work, prioritizing D production.
        pend = None

        def emit_drain(p):
            ms_, ps3_, last = p
            out3 = (out[:, ms_ * CHUNK_F : (ms_ + 1) * CHUNK_F]
                    .rearrange("(lc p) c -> p lc c", p=LCHUNK))
            if not last:
                o_sb = opool.tile([LCHUNK, N_LC * CHUNK_F], BF16)
                o3 = o_sb[:].rearrange("p (lc c) -> p lc c", c=CHUNK_F)
                nc.scalar.copy(o3[:], ps3_)
                nc.sync.dma_start(out3, o3)
            else:  # two ACT halves, DMAs on sync+scalar. (A DVE/ACT split
                # was tried: the framework serializes the second engine's
                # copy behind the first's retirement, so one engine doing
                # both halves back-to-back is strictly better.)
                oa = opool.tile([LCHUNK, 2 * CHUNK_F], BF16)
                ob = opool.tile([LCHUNK, 2 * CHUNK_F], BF16)
                oa3 = oa[:].rearrange("p (lc c) -> p lc c", c=CHUNK_F)
                ob3 = ob[:].rearrange("p (lc c) -> p lc c", c=CHUNK_F)
                nc.scalar.copy(oa3[:], ps3_[:, 0:2])
                nc.sync.dma_start(out3[:, 0:2], oa3)
                nc.scalar.copy(ob3[:], ps3_[:, 2:4])
                nc.scalar.dma_start(out3[:, 2:4], ob3)

        for ms in range(N_MS):
            # D tiles for this block
            dts = []
            pat = D_PATS[D_NV[ms]]
            for j in range(MSUB):
                m = ms * MSUB + j
                dt_ = dpool.tile([H, L], BF16, tag="d")
                if pat[j] == "V":
                    nc.vector.tensor_scalar(
                        dt_[:], x1t_bf, negx2_f[:, m : m + 1], 0.0,
                        op0=mybir.AluOpType.add, op1=mybir.AluOpType.max)
                else:
                    nc.scalar.activation(
                        dt_[:], x1t_bf, mybir.ActivationFunctionType.Relu,
                        bias=negx2_f[:, m : m + 1], scale=1.0)
                dts.append(dt_)

            if ms == 0:
                # v3 bulk rides second on the gpsimd ring; lands ~15.5us,
                # needed by block 2's t3 (~17.5us)
                nc.gpsimd.dma_start(wv3_tile[:, WVA:], wv3[:, WVA:])
            if pend is not None:
                emit_drain(pend)

            ps = psum.tile([LCHUNK, N_LC * PS_STRIDE], F32, tag="ps")
            # t3 (+t1 fold) opens each group full-width (PSUM zeroing is
            # bank-granular), then the t4s accumulate j-major.
            for lc in range(N_LC):
                nc.tensor.matmul(
                    ps[:, lc * PS_STRIDE : lc * PS_STRIDE + CHUNK_F],
                    x1t_bf[:, lc * LCHUNK : (lc + 1) * LCHUNK],
                    v3_bf[:, ms * CHUNK_F : (ms + 1) * CHUNK_F],
                    start=True, stop=False, skip_group_check=True)
            for j in range(MSUB):
                for lc in range(N_LC):
                    base = lc * PS_STRIDE
                    nc.tensor.matmul(
                        ps[:, base + j * C : base + (j + 1) * C],
                        dts[j][:, lc * LCHUNK : (lc + 1) * LCHUNK],
                        w4t2_bf[:],
                        start=False, stop=(j == MSUB - 1),
                        skip_group_check=True)

            ps3 = ps[:].rearrange("p (lc x) -> p lc x",
                                  x=PS_STRIDE)[:, :, 0:CHUNK_F]
            pend = (ms, ps3, ms == N_MS - 1)
        emit_drain(pend)
    return nc


_COMPILED = {}


def _get_compiled():
    if "nc" not in _COMPILED:
        nc = bacc.Bacc("TRN2", target_bir_lowering=False, debug=False,
                       num_devices=N_CORES)
        build_kernel(nc)
        nc.compile()
        _COMPILED["nc"] = nc
    return _COMPILED["nc"]


def make_in_maps(x1, x2, W, b):
    W1, W2, W3, W4 = (W[:, 0:H], W[:, H : 2 * H], W[:, 2 * H : 3 * H],
                      W[:, 3 * H : 4 * H])
    w13 = (W1 - W4).T.astype(np.float32)          # [H, C]
    w3t = W3.T.astype(np.float32)                 # [H, C]
    w4t2 = (2.0 * W4).T.astype(np.float32)        # [H, C]
    in_maps = []
    for cid in range(N_CORES):
        bb, mblk = cid // 4, cid % 4
        m0 = mblk * MB
        x2blk = x2[bb, m0 : m0 + MB]              # [MB, H]
        x2t = x2blk.T                             # [H, MB]
        # V3[h, m*C+c] = x2t[h,m]*W3T[h,c] + (W1-W4)T[h,c]
        v3 = x2t[:, :, None] * w3t[:, None, :] + w13[:, None, :]
        wv3 = np.concatenate([w4t2, v3.reshape(H, MB * C)], axis=1)
        # xin: x1t bf16 cols 0:512 | negx2 f32 bytes viewed as bf16 cols
        # 512:768 (bitcast back to f32 on device)
        xin = np.empty((H, L + 2 * MB), dtype=BF16_NP)
        xin[:, 0:L] = x1[bb].T.astype(BF16_NP)
        negx2 = np.ascontiguousarray(-x2t.astype(np.float32))
        xin[:, L:] = negx2.view(np.uint16).view(BF16_NP).reshape(H, 2 * MB)
        in_maps.append({
            "xin": xin,
            "wv3": np.ascontiguousarray(wv3.astype(BF16_NP)),
        })
    return in_maps


def t2_bias(x2, W, b):
    """Host-side t2 term: x2 @ (W2+W4).T + bias, [B, L, C] f32."""
    W2 = W[:, H : 2 * H]
    W4 = W[:, 3 * H : 4 * H]
    return (x2 @ (W2 + W4).T + b).astype(np.float32)


def run_on_device(x1, x2, W, b, trace=False, trace_kwargs=None):
    nc = _get_compiled()
    in_maps = make_in_maps(x1, x2, W, b)
    res = bass_utils.run_bass_kernel_spmd(
        nc, in_maps, core_ids=list(range(N_CORES)), trace=trace,
        **(trace_kwargs or {}))
    t2 = t2_bias(x2, W, b)                        # [B, L, C]
    full = np.empty((B, L, L, C), dtype=np.float32)
    for cid in range(N_CORES):
        bb, mblk = cid // 4, cid % 4
        m0 = mblk * MB
        full[bb, :, m0 : m0 + MB, :] = (
            np.asarray(res.results[cid]["out"])
            .astype(np.float32).reshape(L, MB, C)
            + t2[bb, m0 : m0 + MB, :][None, :, :])
    return full, res


def kernel(x1, x2, W, b):
    x1 = np.asarray(x1, dtype=np.float32)
    x2 = np.asarray(x2, dtype=np.float32)
    W = np.asarray(W, dtype=np.float32)
    b = np.asarray(b, dtype=np.float32)
    full, _ = run_on_device(x1, x2, W, b, trace=False)
    return full


# revision 21
# speedup vs baseline: 1.1422x; 1.0254x over previous
"""Trainium2 Bass kernel for the BiaffineLayer problem.

Math (per batch b):
  out[l, m, c] = x1[l] @ W1[c] + x2[m] @ W2[c]
              + sum_h x1[l,h] * x2[m,h] * W3[c,h]
              + sum_h |x1[l,h] - x2[m,h]| * W4[c,h] + bias[c]
  shapes: x1, x2 [2, 512, 128]; W [25, 512]; bias [25]; out [2, 512, 512, 25]

Sharding: 8 cores = 2 batches x 4 m-blocks of 128 columns. Core (b, mb) gets
full x1[b] and its x2[b, m0:m0+128] block; it produces out[b, :, m0:m0+MB, :].

Decomposition, with |d| = 2*relu(d) - d and d = x1 - x2[m]:
  out = x1t' @ V3  +  D_m' @ (2 W4T)  +  T2B
where (host-precomputed except D):
  V3[h,(m,c)] = x2[m,h]*W3[c,h] + (W1-W4)[c,h]   (t3 + t1 - t4's -x1*W4 part)
  T2B[m,c]    = x2[m] @ (W2+W4)T + b             (added on the HOST during
                                                  unshard - pure per-(m,c))
  D_m[h,l]    = relu(x1[l,h] - x2[m,h])          (device, pairwise)

Measured HW rates (microbenched): DVE tensor_scalar = 129ns + 0.26ns/elem
(~263ns per [128,512] D tile, 2x bf16 mode already engaged); ACT relu ~612ns
effective, ACT [128,1600] PSUM drain ~1480ns; PE t4 LDW+MM pair 25ns, t3
400-col MM 168ns (PE total ~18us, not critical); input DMA is descriptor-
rate-bound (~17ns per 1KB partition row -> 2.2us for x1t on one queue);
gpsimd compute useless (7.5us/op); large out-DMAs on the gpsimd queue hit a
~19us descriptor pathology - keep them on sync.

Schedule (v3):
  - input DMAs split across queues to parallelize descriptor processing:
    sync: x1t[:,0:256] -> wv3a(w4t2+v3 blocks 0-1) -> out-DMAs
    vector: x1t[:,256:512] -> wv3b(v3 blocks 2-4)
    gpsimd: negx2[:,0:64] -> negx2[:,64:128] -> wv3c(v3 blocks 5-7)
    D-gen starts ~9.1us instead of ~10.6.
  - single 4-bank PSUM tile per block (bufs=2), drain = ONE ACT op
    [128,4,400] per block, emitted one block late; last block's drain
    split DVE+ACT with its two out-DMAs on sync+tensor queues.
  - D-tile engine split DVE 102 / ACT 26 (block 0-1 give ACT 4 tiles as
    they carry no/less drain work).
"""

import sys

sys.path.insert(0, "/opt/trn_rl_repo")

from contextlib import ExitStack

import ml_dtypes
import numpy as np

import concourse.bass as bass
import concourse.tile as tile
from concourse import bacc, bass_utils, mybir

F32 = mybir.dt.float32
BF16 = mybir.dt.bfloat16
BF16_NP = ml_dtypes.bfloat16

B, L, H, C = 2, 512, 128, 25
MB = 128            # m-block per core
N_CORES = 8
MSUB = 16           # m's per psum block
N_MS = MB // MSUB   # 8 blocks over the m-block
LCHUNK = 128
N_LC = L // LCHUNK  # 4 l-chunks
CHUNK_F = MSUB * C  # 400 psum free columns per l-chunk slice
PS_STRIDE = 512     # psum bank stride (f32 elems) per l-chunk slice

# Per-block D-tile engine split: V=DVE, A=ACT. sum V = 108.
# ACT's per-block queue is [A-tiles..., drain(k-1)]; A positions are EARLY
# (j=1,3,..) so the block's end never waits on the slower ACT engine --
# the cadence is then set purely by DVE (nV x 263ns). The drain runs in
# ACT's idle tail of each block. First and last blocks are ACT-heavy:
# block 0 has no drain yet, and a DVE-light final block shortens the tail
# (DVE finishes early and takes half of the final drain).
D_PATS = {
    12: "VAVAVAVAVVVVVVVV",   # V=12 A=4 at j=1,3,5,7
    14: "VAVAVVVVVVVVVVVV",   # V=14 A=2 at j=1,3
}
D_NV = [12, 14, 14, 14, 14, 14, 14, 12]   # sum = 108


def build_kernel(nc: bass.Bass, repeat: int = 1):
    # xin = x1t bf16 [H, 512] cols 0:512 | negx2 f32 [H, 128] as bf16-viewed
    # bytes in cols 512:768. Merging rides negx2's bytes on x1t's fat
    # 1536B rows: one DMA per partition-half at full bandwidth instead of
    # a separate 512B-row DMA that measured ~5x slower and landed last.
    xin = nc.dram_tensor("xin", (H, L + 2 * MB), BF16, kind="ExternalInput").ap()
    # w4t2 (25 cols) | v3 (3200 cols) merged
    wv3 = nc.dram_tensor("wv3", (H, C + MB * C), BF16, kind="ExternalInput").ap()
    out = nc.dram_tensor("out", (L, MB * C), BF16, kind="ExternalOutput").ap()

    WVA = C + 2 * CHUNK_F        # w4t2 + v3 blocks 0-1

    with tile.TileContext(nc) as tc, ExitStack() as ctx:
      const = ctx.enter_context(tc.tile_pool(name="const", bufs=1))
      dpool = ctx.enter_context(tc.tile_pool(name="dpool", bufs=128))
      opool = ctx.enter_context(tc.tile_pool(name="opool", bufs=10))
      psum = ctx.enter_context(tc.tile_pool(name="psum", bufs=4, space="PSUM"))
      for _rep in range(repeat):
        # ---- input loads ----
        # DMA queues are sync/scalar/gpsimd only. A DMA costs ~1.3us
        # fixed startup + transfer at ~230GB/s for >=1KB rows, and a
        # ring's SECOND DMA starts ~1us after its first finishes. So
        # everything D-gen needs rides FIRST on a ring: xin halves
        # (partition split) on sync+scalar, w4t2+v3[0:2] on gpsimd.
        # The scalar ring processes its half while the ACT engine loads
        # the activation table (act_warm), costing ACT nothing.
        xin_tile = const.tile([H, L + 2 * MB], BF16)
        x1t_bf = xin_tile[:, 0:L]
        negx2_f = xin_tile[:, L:].bitcast(F32)
        wv3_tile = const.tile([H, C + MB * C], BF16)
        w4t2_bf = wv3_tile[:, 0:C]
        v3_bf = wv3_tile[:, C:]
        ones_bf = const.tile([1, LCHUNK], BF16)

        nc.scalar.dma_start(xin_tile[64:128, :], xin[64:128, :])
        nc.sync.dma_start(xin_tile[0:64, :], xin[0:64, :])
        nc.gpsimd.dma_start(wv3_tile[:, 0:WVA], wv3[:, 0:WVA])
        nc.vector.memset(ones_bf[:], 1.0)
        # preload the ACT activation table off the critical path
        act_warm = const.tile([1, LCHUNK], BF16)
        nc.scalar.activation(act_warm[:], ones_bf[:],
                             mybir.ActivationFunctionType.Relu)

        # ---- main loop over m-blocks ----
        # Drains (one ACT op each) emitted one block late so they queue
        # behind the next block's D work, prioritizing D production.
        pend = None

        def emit_drain(p):
            # psa3_/psb3_ are SEPARATE psum tiles: cross-engine readers of
            # one tile serialize in the framework, so the final block's
            # DVE+ACT halves only run in parallel with two tiles.
            ms_, psa3_, psb3_, last = p
            out3 = (out[:, ms_ * CHUNK_F : (ms_ + 1) * CHUNK_F]
                    .rearrange("(lc p) c -> p lc c", p=LCHUNK))
            o_sb = opool.tile([LCHUNK, N_LC * CHUNK_F], BF16)
            o3 = o_sb[:].rearrange("p (lc c) -> p lc c", c=CHUNK_F)
            if not last:
                nc.scalar.copy(o3[:, 0:2], psa3_)
                nc.scalar.copy(o3[:, 2:4], psb3_)
                nc.sync.dma_start(out3, o3)
            else:  # split engines + 2 DMAs on 2 queues for a short tail
                nc.vector.tensor_copy(o3[:, 0:2], psa3_)
                nc.sync.dma_start(out3[:, 0:2], o3[:, 0:2])
                nc.scalar.copy(o3[:, 2:4], psb3_)
                nc.scalar.dma_start(out3[:, 2:4], o3[:, 2:4])

        for ms in range(N_MS):
            # D tiles for this block
            dts = []
            pat = D_PATS[D_NV[ms]]
            for j in range(MSUB):
                m = ms * MSUB + j
                dt_ = dpool.tile([H, L], BF16, tag="d")
                if pat[j] == "V":
                    nc.vector.tensor_scalar(
                        dt_[:], x1t_bf, negx2_f[:, m : m + 1], 0.0,
                        op0=mybir.AluOpType.add, op1=mybir.AluOpType.max)
                else:
                    nc.scalar.activation(
                        dt_[:], x1t_bf, mybir.ActivationFunctionType.Relu,
                        bias=negx2_f[:, m : m + 1], scale=1.0)
                dts.append(dt_)

            if ms == 0:
                # v3 bulk rides second on the gpsimd ring; lands ~15.5us,
                # needed by block 2's t3 (~17.5us)
                nc.gpsimd.dma_start(wv3_tile[:, WVA:], wv3[:, WVA:])
            if pend is not None:
                emit_drain(pend)

            ps_a = psum.tile([LCHUNK, 2 * PS_STRIDE], F32, tag="ps")
            ps_b = psum.tile([LCHUNK, 2 * PS_STRIDE], F32, tag="ps")
            pss = [ps_a, ps_a, ps_b, ps_b]
            # t3 (+t1 fold) opens each group full-width (PSUM zeroing is
            # bank-granular), then the t4s accumulate j-major.
            for lc in range(N_LC):
                nc.tensor.matmul(
                    pss[lc][:, (lc % 2) * PS_STRIDE :
                            (lc % 2) * PS_STRIDE + CHUNK_F],
                    x1t_bf[:, lc * LCHUNK : (lc + 1) * LCHUNK],
                    v3_bf[:, ms * CHUNK_F : (ms + 1) * CHUNK_F],
                    start=True, stop=False, skip_group_check=True)
            for j in range(MSUB):
                for lc in range(N_LC):
                    base = (lc % 2) * PS_STRIDE
                    nc.tensor.matmul(
                        pss[lc][:, base + j * C : base + (j + 1) * C],
                        dts[j][:, lc * LCHUNK : (lc + 1) * LCHUNK],
                        w4t2_bf[:],
                        start=False, stop=(j == MSUB - 1),
                        skip_group_check=True)

            psa3 = ps_a[:].rearrange("p (lc x) -> p lc x",
                                     x=PS_STRIDE)[:, :, 0:CHUNK_F]
            psb3 = ps_b[:].rearrange("p (lc x) -> p lc x",
                                     x=PS_STRIDE)[:, :, 0:CHUNK_F]
            pend = (ms, psa3, psb3, ms == N_MS - 1)
        emit_drain(pend)
    return nc


_COMPILED = {}


def _get_compiled():
    if "nc" not in _COMPILED:
        nc = bacc.Bacc("TRN2", target_bir_lowering=False, debug=False,
                       num_devices=N_CORES)
        build_kernel(nc)
        nc.compile()
        _COMPILED["nc"] = nc
    return _COMPILED["nc"]


def make_in_maps(x1, x2, W, b):
    W1, W2, W3, W4 = (W[:, 0:H], W[:, H : 2 * H], W[:, 2 * H : 3 * H],
                      W[:, 3 * H : 4 * H])
    w13 = (W1 - W4).T.astype(np.float32)          # [H, C]
    w3t = W3.T.astype(np.float32)                 # [H, C]
    w4t2 = (2.0 * W4).T.astype(np.float32)        # [H, C]
    in_maps = []
    for cid in range(N_CORES):
        bb, mblk = cid // 4, cid % 4
        m0 = mblk * MB
        x2blk = x2[bb, m0 : m0 + MB]              # [MB, H]
        x2t = x2blk.T                             # [H, MB]
        # V3[h, m*C+c] = x2t[h,m]*W3T[h,c] + (W1-W4)T[h,c]
        v3 = x2t[:, :, None] * w3t[:, None, :] + w13[:, None, :]
        wv3 = np.concatenate([w4t2, v3.reshape(H, MB * C)], axis=1)
        # xin: x1t bf16 cols 0:512 | negx2 f32 bytes viewed as bf16 cols
        # 512:768 (bitcast back to f32 on device)
        xin = np.empty((H, L + 2 * MB), dtype=BF16_NP)
        xin[:, 0:L] = x1[bb].T.astype(BF16_NP)
        negx2 = np.ascontiguousarray(-x2t.astype(np.float32))
        xin[:, L:] = negx2.view(np.uint16).view(BF16_NP).reshape(H, 2 * MB)
        in_maps.append({
            "xin": xin,
            "wv3": np.ascontiguousarray(wv3.astype(BF16_NP)),
        })
    return in_maps


def t2_bias(x2, W, b):
    """Host-side t2 term: x2 @ (W2+W4).T + bias, [B, L, C] f32."""
    W2 = W[:, H : 2 * H]
    W4 = W[:, 3 * H : 4 * H]
    return (x2 @ (W2 + W4).T + b).astype(np.float32)


def run_on_device(x1, x2, W, b, trace=False, trace_kwargs=None):
    nc = _get_compiled()
    in_maps = make_in_maps(x1, x2, W, b)
    res = bass_utils.run_bass_kernel_spmd(
        nc, in_maps, core_ids=list(range(N_CORES)), trace=trace,
        **(trace_kwargs or {}))
    t2 = t2_bias(x2, W, b)                        # [B, L, C]
    full = np.empty((B, L, L, C), dtype=np.float32)
    for cid in range(N_CORES):
        bb, mblk = cid // 4, cid % 4
        m0 = mblk * MB
        full[bb, :, m0 : m0 + MB, :] = (
            np.asarray(res.results[cid]["out"])
            .astype(np.float32).reshape(L, MB, C)
            + t2[bb, m0 : m0 + MB, :][None, :, :])
    return full, res


def kernel(x1, x2, W, b):
    x1 = np.asarray(x1, dtype=np.float32)
    x2 = np.asarray(x2, dtype=np.float32)
    W = np.asarray(W, dtype=np.float32)
    b = np.asarray(b, dtype=np.float32)
    full, _ = run_on_device(x1, x2, W, b, trace=False)
    return full
